# revision 14
# baseline (speedup 1.0000x reference)
"""Multi-head attention (raw-reshape variant) on 8 trn2 NeuronCores.

Shapes: B=2, S=2048, D=1024, H=16, dh=64.  The reference uses a raw
reshape (B,S,D)->(B,H,S,dh) (NOT a head transpose), so head h only sees
projected rows [128h, 128h+128).  Each (b, h) pair is fully independent:
core c handles b=c//4 and the 4 heads of seq-block c%4.  No collectives.

Folded index convention per pair (128 input rows r, 1024 dims):
  s' = 16 r + t,  dm = 64 t + d   (t in [0,16), d in [0,64))
The kernel works in a PERMUTED query order q'' = 128 t + r (t-major);
the host permutes the mask columns to match and the final projection
un-permutes for free.

Schedule (per core).  The attention inner loop is ACT-engine bound
(exp of 16.8M scores = 128 x ~1.15us); the tensor engine has slack per
step, which is filled with deferred output-projection matmuls so the PE
never idles.  The prologue is DMA-bandwidth bound, so mask loads are
staggered: 3 tiles + w_o up front, the rest issued one per attention
step ~6 steps ahead of first use, keeping the early HBM window clear
for the q/k/v chunk loads.

  prologue: Q projection with M=128 psum tiles ([2 t-blocks x dh] on
    partitions, 8 banks); scatter into qt via 2 same-partition DVE
    copies plus 2 cross-partition SBUF->SBUF DMAs per tile (issued on
    the idle ACT engine's HWDGE queue).  K projection in two 4-bank
    passes and V projection in 8 one-bank groups, both interleaved with
    the first 10 attention steps (S/exp/mask only) so the ACT engine
    starts ~17us in.  Exp act-table warmed with a dummy activation.
    w_o loaded once into persistent SBUF (wo3 pairs t with t+8 on
    partition halves).
  attention: 8 phases (pair, q-half) x 16 t-steps in one flat software
    pipeline: S matmuls (fp16, K=64) -> exp on ACT (scale=1/8 fused) ->
    mask multiply on DVE -> PV matmuls trailing by LAG.  Vaug per-t
    blocks are [ones | V_t] so PV also emits 64 broadcast rows of the
    softmax denominator (psO rows 0:63), O^T on rows 64:127.
  normalize: DVE reciprocal + mul into stack3[p] (qh0 O to partitions
    0:63 via tmp + partition-move DMA, qh1 direct to 64:127), deferred
    one step so it never delays pm production at phase boundaries.
  final: per pair, 16 matmuls psF += stack3[:,128u:+128].T @ wo3[u]
    (K=128: dh of block u on 0:63 + block u+8 on 64:127), interleaved
    into the next phase's steps; psF -> outsb copy on DVE -> DRAM.
"""

import numpy as np

import concourse.bass as bass
import concourse.mybir as mybir
import concourse.tile as tile
from concourse import bacc
from concourse.bass_utils import run_bass_kernel_spmd

F32 = mybir.dt.float32
F16 = mybir.dt.float16

B, S, D, H, DH = 2, 2048, 1024, 16, 64
N_CORES = 8
CORE_ROWS = 512          # seq rows per core
N_PAIRS = 4              # (b, h) pairs per core
EXP_SCALE = 0.125        # 1/sqrt(dh)
LAG = 2
N_EARLY = 10             # attention steps interleaved into K/V assembly
MASK_UPFRONT = 5

_NC = None


def _build_program():
    nc = bacc.Bacc()

    # host-concatenated [w_chunk | x_chunk] per contraction chunk k
    qasm = nc.dram_tensor("qasm", [8, 128, 1536], F16, kind="ExternalInput")
    kasm = nc.dram_tensor("kasm", [8, 128, 1536], F16, kind="ExternalInput")
    vasm = nc.dram_tensor("vasm", [8, 128, 1536], F16, kind="ExternalInput")
    wo3_d = nc.dram_tensor("wo3", [8, 128, D], F16, kind="ExternalInput")
    maskc_d = nc.dram_tensor("maskc", [S, S], F16, kind="ExternalInput")
    out_d = nc.dram_tensor("out", [CORE_ROWS, D], F32, kind="ExternalOutput")

    with tile.TileContext(nc) as tc:
        with tc.tile_pool(name="persist", bufs=1) as persist:
            maskc_sb = [persist.tile([128, S], F16, tag=f"mask{t}", name=f"mask{t}")
                        for t in range(16)]

            # col = 2048 g + 128 t + r  (q''-order, group-major)
            qt_all = persist.tile([128, 2 * S], F16, tag="qt", name="qt")
            kt_all = persist.tile([128, 2 * S], F16, tag="kt", name="kt")
            qt = [qt_all[:, S * g:S * (g + 1)] for g in range(2)]
            kt = [kt_all[:, S * g:S * (g + 1)] for g in range(2)]
            vaug = [persist.tile([128, 2048], F16, tag=f"vaug{p}", name=f"vaug{p}")
                    for p in range(N_PAIRS)]
            # per-t blocks are [ones (64) | V_t (64)]: denominator rows land
            # at psO partitions 0:63 (reciprocal_approx_fast needs base 0)
            for p in range(N_PAIRS):
                va3 = vaug[p][:, :].rearrange("p (t c) -> p t c", c=128)
                nc.vector.memset(va3[:, :, 0:64], 1.0)
            # stack3[p]: col = 128 u + r; partitions 0:63 = O(t=u) dh,
            # 64:127 = O(t=u+8) dh  (qh0 / qh1 halves of the fold)
            stack3 = [persist.tile([128, 1024], F16, tag=f"stk{p}", name=f"stk{p}")
                      for p in range(N_PAIRS)]
            wo_sb = [persist.tile([128, D], F16, tag=f"wo{u}", name=f"wo{u}")
                     for u in range(8)]
            dumm = persist.tile([128, 8], F32, tag="dumm", name="dumm")

            # warm the exp table set while the PE does assembly
            nc.scalar.activation(dumm[:, :], dumm[:, :],
                                 mybir.ActivationFunctionType.Exp, scale=1.0)

            mask_issued = [False] * 16

            def mask_load(t):
                if not mask_issued[t]:
                    mask_issued[t] = True
                    nc.sync.dma_start(out=maskc_sb[t][:, :],
                                      in_=maskc_d[t::16, :])

            with tc.tile_pool(name="outb", bufs=2) as outpool, \
                 tc.tile_pool(name="pr_pool", bufs=5) as prpool, \
                 tc.tile_pool(name="pm_pool", bufs=12) as pmpool, \
                 tc.tile_pool(name="asm_mt", bufs=8) as mtpool, \
                 tc.tile_pool(name="asm_tmp", bufs=3) as tmppool:

                # ---------------- projections (M=128 psum tiles) ---------
                # psum tile u covers w-cols 128u..128u+128 = fold blocks
                # (2u, 2u+1): partitions 0:63 = dh@t=2u, 64:127 = dh@t=2u+1;
                # cols = 512 x-rows = 4 pair blocks (g,hp).
                def scatter_qk(ps, dst_all, u):
                    def psv(a):  # [64, 4, 128] view of partition half a
                        return ps[64 * a:64 * (a + 1), :].rearrange(
                            "p (q r) -> p q r", r=128)

                    def dstv(hp, t):  # [64, 2, 128] dst: both g, block t
                        return dst_all[64 * hp:64 * (hp + 1),
                                       :].rearrange("p (g c) -> p g c",
                                                    g=2)[:, :, 128 * t:128 * (t + 1)]

                    tA, tB = 2 * u, 2 * u + 1
                    # same-partition pieces (t-parity == hp)
                    nc.vector.tensor_copy(dstv(0, tA), psv(0)[:, 0::2, :])
                    nc.vector.tensor_copy(dstv(1, tB), psv(1)[:, 1::2, :])
                    # cross pieces via fp16 tmp + partition-move DMA
                    tmp = tmppool.tile([128, 256], F16, tag="xt", name="xt")
                    t3a = tmp[0:64, :].rearrange("p (q r) -> p q r", r=128)
                    t3b = tmp[64:128, :].rearrange("p (q r) -> p q r", r=128)
                    nc.vector.tensor_copy(t3a, psv(0)[:, 1::2, :])
                    nc.gpsimd.dma_start(out=dstv(1, tA), in_=t3a)
                    nc.vector.tensor_copy(t3b, psv(1)[:, 0::2, :])
                    nc.gpsimd.dma_start(out=dstv(0, tB), in_=t3b)

                # Q assembly first (attention needs all of qt)
                mts_q = []
                for k in range(8):
                    mt = mtpool.tile([128, 1536], F16, tag="mt", name="mtq")
                    nc.sync.dma_start(out=mt[:, :], in_=qasm[k])
                    mts_q.append(mt)
                mts_k = []
                for k in range(8):
                    mt = mtpool.tile([128, 1536], F16, tag="mt", name="mtk")
                    nc.sync.dma_start(out=mt[:, :], in_=kasm[k])
                    mts_k.append(mt)
                for t in range(MASK_UPFRONT):
                    mask_load(t)

                with tc.tile_pool(name="asm_psq", bufs=8, space="PSUM") as apsq:
                    ps8 = [apsq.tile([128, 512], F32, tag="asm", name="asmps")
                           for _ in range(8)]
                    for k in range(8):
                        for u in range(8):
                            nc.tensor.matmul(
                                ps8[u][:, :],
                                lhsT=mts_q[k][:, 128 * u:128 * (u + 1)],
                                rhs=mts_q[k][:, 1024:1536],
                                start=(k == 0), stop=(k == 7),
                            )
                    for u in range(8):
                        scatter_qk(ps8[u], qt_all, u)

                # ---------------- attention pipeline state ---------------
                flat = [(p, qh, t) for p in range(N_PAIRS)
                        for qh in range(2) for t in range(16)]
                queue = []      # S-emitted steps awaiting PV
                extra = []      # deferred PE work (final-projection units)
                dve_extra = []  # deferred DVE work (normalize closures)
                psO_map = {}
                step_no = [0]

                with tc.tile_pool(name="st_ps", bufs=2, space="PSUM") as stpsum:

                    def emit_s_step():
                        p, qh, t = flat[step_no[0]]
                        # stagger the remaining mask loads ~5 steps ahead
                        mask_load(min(step_no[0] + MASK_UPFRONT, 15))
                        step_no[0] += 1
                        g, hp = p // 2, p % 2
                        lo, hi = 64 * hp, 64 * (hp + 1)
                        stt = stpsum.tile([128, 1024], F32, tag="st", name="stt")
                        for sc in range(2):
                            nc.tensor.matmul(
                                stt[:, 512 * sc:512 * (sc + 1)],
                                lhsT=kt[g][lo:hi, 128 * t:128 * (t + 1)],
                                rhs=qt[g][lo:hi,
                                          1024 * qh + 512 * sc:
                                          1024 * qh + 512 * (sc + 1)],
                                start=True, stop=True,
                            )
                        praw = prpool.tile([128, 1024], F16, tag="praw",
                                           name="praw")
                        nc.scalar.activation(praw[:, :], stt[:, :],
                                             mybir.ActivationFunctionType.Exp,
                                             scale=EXP_SCALE)
                        pm = pmpool.tile([128, 1024], F16, tag="pm", name="pm")
                        nc.vector.tensor_mul(
                            pm[:, :], praw[:, :],
                            maskc_sb[t][:, 1024 * qh:1024 * (qh + 1)])
                        queue.append((p, qh, t, pm))

                    # K assembly in two 4-bank passes, early steps between
                    with tc.tile_pool(name="asm_psk", bufs=4,
                                      space="PSUM") as apsk:
                        for half in range(2):
                            ps4 = [apsk.tile([128, 512], F32, tag="asmk",
                                             name="asmk") for _ in range(4)]
                            for k in range(8):
                                for ui in range(4):
                                    u = 4 * half + ui
                                    nc.tensor.matmul(
                                        ps4[ui][:, :],
                                        lhsT=mts_k[k][:, 128 * u:128 * (u + 1)],
                                        rhs=mts_k[k][:, 1024:1536],
                                        start=(k == 0), stop=(k == 7),
                                    )
                            for ui in range(4):
                                scatter_qk(ps4[ui], kt_all, 4 * half + ui)
                            # steps 0-2 need kt blocks 0-5, covered by
                            # pass 1 (u-tiles 0-3 = blocks 0-7)
                            for _ in range(3):
                                emit_s_step()

                    mtv = []
                    for k in range(8):
                        mt = mtpool.tile([128, 1536], F16, tag="mt", name="mtv")
                        nc.sync.dma_start(out=mt[:, :], in_=vasm[k])
                        mtv.append(mt)
                    for u in range(8):
                        nc.sync.dma_start(out=wo_sb[u][:, :], in_=wo3_d[u])

                    def v_scatter(ps, p, oc):
                        src = ps[:, :].rearrange("p (t c) -> p t c", c=64)
                        dst3 = vaug[p][:, :].rearrange("p (t c) -> p t c", c=128)
                        nc.vector.tensor_copy(
                            dst3[:, 8 * oc:8 * (oc + 1), 64:128], src)

                    # V projection, 8 one-bank groups, early steps between
                    with tc.tile_pool(name="v_ps", bufs=4, space="PSUM") as vpsum:
                        for p in range(N_PAIRS):
                            for oc in range(2):
                                psv = vpsum.tile([128, 512], F32, tag="psv",
                                                 name="psv")
                                for k in range(8):
                                    nc.tensor.matmul(
                                        psv[:, :],
                                        lhsT=mtv[k][:, 1024 + 128 * p:
                                                    1024 + 128 * (p + 1)],
                                        rhs=mtv[k][:, 512 * oc:512 * (oc + 1)],
                                        start=(k == 0), stop=(k == 7),
                                    )
                                v_scatter(psv, p, oc)
                                if step_no[0] < N_EARLY and (p + oc) % 2 == 1:
                                    emit_s_step()

                    with tc.tile_pool(name="norm", bufs=2) as npool, \
                         tc.tile_pool(name="o_ps", bufs=2, space="PSUM") as opsum:

                        def final_unit(p, i):
                            # i = 4u + 2oc + h: quarter-bank N=256 units so
                            # one unit per step stays under the ACT cadence
                            if i == 0:
                                psO_map[("F", p)] = opsum.tile(
                                    [128, 1024], F32, tag="o", name="psF")
                            psF = psO_map[("F", p)]
                            u, oc, h = i // 4, (i // 2) % 2, i % 2
                            c0 = 512 * oc + 256 * h
                            nc.tensor.matmul(
                                psF[:, c0:c0 + 256],
                                lhsT=stack3[p][:, 128 * u:128 * (u + 1)],
                                rhs=wo_sb[u][:, c0:c0 + 256],
                                start=(u == 0 and h == 0), stop=(u == 7),
                            )
                            if i == 31:
                                psF = psO_map.pop(("F", p))
                                ob = outpool.tile([128, D], F32, tag="ob",
                                                  name="ob")
                                nc.vector.tensor_copy(ob[:, :], psF[:, :])
                                nc.gpsimd.dma_start(
                                    out=out_d[128 * p:128 * (p + 1), :],
                                    in_=ob[:, :])

                        def normalize(p, qh, psO):
                            recip = npool.tile([64, 1024], F32, tag="recip",
                                               name="recip")
                            nc.vector.reciprocal_approx_fast(recip[:, :],
                                                             psO[0:64, :])
                            if qh == 0:
                                tmpa = npool.tile([128, 1024], F16, tag="tmpa",
                                                  name="tmpa")
                                nc.vector.tensor_mul(tmpa[64:128, :],
                                                     psO[64:128, :],
                                                     recip[:, :])
                                nc.gpsimd.dma_start(out=stack3[p][0:64, :],
                                                    in_=tmpa[64:128, :])
                            else:
                                nc.vector.tensor_mul(stack3[p][64:128, :],
                                                     psO[64:128, :],
                                                     recip[:, :])
                                for i in range(32):
                                    extra.append(
                                        lambda p=p, i=i: final_unit(p, i))

                        def drain_one():
                            p, qh, t, pm = queue.pop(0)
                            if t == 0:
                                psO_map[(p, qh)] = opsum.tile(
                                    [128, 1024], F32, tag="o", name="psO")
                            psO = psO_map[(p, qh)]
                            for sc in range(2):
                                nc.tensor.matmul(
                                    psO[:, 512 * sc:512 * (sc + 1)],
                                    lhsT=vaug[p][:, 128 * t:128 * (t + 1)],
                                    rhs=pm[:, 512 * sc:512 * (sc + 1)],
                                    start=(t == 0), stop=(t == 15),
                                )
                            if t == 15:
                                psO = psO_map.pop((p, qh))
                                dve_extra.append(
                                    lambda p=p, qh=qh, psO=psO:
                                    normalize(p, qh, psO))

                        while step_no[0] < len(flat):
                            emit_s_step()
                            if extra:
                                extra.pop(0)()
                            drained = 0
                            while len(queue) > 6 and drained < 2:
                                drain_one()
                                drained += 1
                            if len(queue) > LAG and drained == 0:
                                drain_one()
                            if dve_extra:
                                dve_extra.pop(0)()
                        while queue:
                            drain_one()
                        while dve_extra:
                            dve_extra.pop(0)()
                        while extra:
                            extra.pop(0)()

    nc.finalize()
    return nc


def build_in_maps(inputs):
    q = np.asarray(inputs["q"], dtype=np.float32)
    k = np.asarray(inputs["k"], dtype=np.float32)
    v = np.asarray(inputs["v"], dtype=np.float32)
    mask = np.asarray(inputs["mask"])
    w_q = np.asarray(inputs["w_q"], dtype=np.float32)
    w_k = np.asarray(inputs["w_k"], dtype=np.float32)
    w_v = np.asarray(inputs["w_v"], dtype=np.float32)
    w_o = np.asarray(inputs["w_o"], dtype=np.float32)

    wqT = np.ascontiguousarray(w_q.T).astype(np.float16).reshape(8, 128, D)
    wkT = np.ascontiguousarray(w_k.T).astype(np.float16).reshape(8, 128, D)
    wvT = np.ascontiguousarray(w_v.T).astype(np.float16).reshape(8, 128, D)
    woT = np.ascontiguousarray(w_o.T)
    wo3 = np.stack([np.concatenate([woT[64 * u:64 * u + 64],
                                    woT[512 + 64 * u:512 + 64 * u + 64]], axis=0)
                    for u in range(8)]).astype(np.float16)
    # St rows are k'; columns are q'' = 128 t + r (permuted query order):
    # maskc[k', 128 t + r] = 1 - mask[b][q' = 16 r + t, k']
    maskc = []
    for b in range(B):
        mt_ = (~mask[b]).T.astype(np.float16)          # [k', q']
        mp = mt_.reshape(S, 128, 16).transpose(0, 2, 1).reshape(S, S)
        maskc.append(np.ascontiguousarray(mp))

    in_maps = []
    for c in range(N_CORES):
        b, sb = c // 4, c % 4
        rows = slice(CORE_ROWS * sb, CORE_ROWS * (sb + 1))
        xqT = np.ascontiguousarray(q[b, rows].T).astype(np.float16).reshape(8, 128, CORE_ROWS)
        xkT = np.ascontiguousarray(k[b, rows].T).astype(np.float16).reshape(8, 128, CORE_ROWS)
        xvT = np.ascontiguousarray(v[b, rows].T).astype(np.float16).reshape(8, 128, CORE_ROWS)
        in_maps.append({
            "qasm": np.concatenate([wqT, xqT], axis=2),
            "kasm": np.concatenate([wkT, xkT], axis=2),
            "vasm": np.concatenate([wvT, xvT], axis=2),
            "wo3": wo3,
            "maskc": maskc[b],
        })
    return in_maps


def kernel(q, k, v, mask, w_q, w_k, w_v, w_o):
    global _NC
    if _NC is None:
        _NC = _build_program()

    in_maps = build_in_maps(dict(q=q, k=k, v=v, mask=mask,
                                 w_q=w_q, w_k=w_k, w_v=w_v, w_o=w_o))
    res = run_bass_kernel_spmd(_NC, in_maps, list(range(N_CORES))).results

    out = np.empty((B, S, D), dtype=np.float32)
    for c in range(N_CORES):
        b, sb = c // 4, c % 4
        out[b, CORE_ROWS * sb:CORE_ROWS * (sb + 1)] = res[c]["out"]
    return out


# revision 15
# speedup vs baseline: 1.0554x; 1.0554x over previous
"""Multi-head attention (raw-reshape variant) on 8 trn2 NeuronCores.

Shapes: B=2, S=2048, D=1024, H=16, dh=64.  The reference uses a raw
reshape (B,S,D)->(B,H,S,dh) (NOT a head transpose), so head h only sees
projected rows [128h, 128h+128).  Each (b, h) pair is fully independent:
core c handles b=c//4 and the 4 heads of seq-block c%4.  No collectives.

Folded index convention per pair (128 input rows r, 1024 dims):
  s' = 16 r + t,  dm = 64 t + d   (t in [0,16), d in [0,64))
The kernel works in a PERMUTED query order q'' = 128 t + r (t-major);
the host permutes the mask columns to match and the final projection
un-permutes for free.

Schedule (per core).  The attention inner loop is ACT-engine bound
(exp of 16.8M scores = 128 x ~1.15us); the tensor engine has slack per
step, which is filled with deferred output-projection matmuls so the PE
never idles.  The prologue is DMA-bandwidth bound, so mask loads are
staggered: 3 tiles + w_o up front, the rest issued one per attention
step ~6 steps ahead of first use, keeping the early HBM window clear
for the q/k/v chunk loads.

  prologue: Q projection with M=128 psum tiles ([2 t-blocks x dh] on
    partitions, 8 banks); scatter into qt via 2 same-partition DVE
    copies plus 2 cross-partition SBUF->SBUF DMAs per tile (issued on
    the idle ACT engine's HWDGE queue).  K projection in two 4-bank
    passes and V projection in 8 one-bank groups, both interleaved with
    the first 10 attention steps (S/exp/mask only) so the ACT engine
    starts ~17us in.  Exp act-table warmed with a dummy activation.
    w_o loaded once into persistent SBUF (wo3 pairs t with t+8 on
    partition halves).
  attention: 8 phases (pair, q-half) x 16 t-steps in one flat software
    pipeline: S matmuls (fp16, K=64) -> exp on ACT (scale=1/8 fused) ->
    mask multiply on DVE -> PV matmuls trailing by LAG.  Vaug per-t
    blocks are [ones | V_t] so PV also emits 64 broadcast rows of the
    softmax denominator (psO rows 0:63), O^T on rows 64:127.
  normalize: DVE reciprocal + mul into stack3[p] (qh0 O to partitions
    0:63 via tmp + partition-move DMA, qh1 direct to 64:127), deferred
    one step so it never delays pm production at phase boundaries.
  final: per pair, 16 matmuls psF += stack3[:,128u:+128].T @ wo3[u]
    (K=128: dh of block u on 0:63 + block u+8 on 64:127), interleaved
    into the next phase's steps; psF -> outsb copy on DVE -> DRAM.
"""

import numpy as np

import concourse.bass as bass
import concourse.mybir as mybir
import concourse.tile as tile
from concourse import bacc
from concourse.bass_utils import run_bass_kernel_spmd

F32 = mybir.dt.float32
F16 = mybir.dt.float16

B, S, D, H, DH = 2, 2048, 1024, 16, 64
N_CORES = 8
CORE_ROWS = 512          # seq rows per core
N_PAIRS = 4              # (b, h) pairs per core
EXP_SCALE = 0.125        # 1/sqrt(dh)
LAG = 2
N_EARLY = 10             # attention steps interleaved into K/V assembly
MASK_UPFRONT = 5

_NC = None


def _build_program():
    nc = bacc.Bacc()

    # host-concatenated [w_chunk | x_chunk] per contraction chunk k
    qasm = nc.dram_tensor("qasm", [8, 128, 1536], F16, kind="ExternalInput")
    kasm = nc.dram_tensor("kasm", [8, 128, 1536], F16, kind="ExternalInput")
    vasm = nc.dram_tensor("vasm", [8, 128, 1536], F16, kind="ExternalInput")
    wo3_d = nc.dram_tensor("wo3", [8, 128, D], F16, kind="ExternalInput")
    maskc_d = nc.dram_tensor("maskc", [S, S], F16, kind="ExternalInput")
    out_d = nc.dram_tensor("out", [CORE_ROWS, D], F32, kind="ExternalOutput")

    with tile.TileContext(nc) as tc:
        with tc.tile_pool(name="persist", bufs=1) as persist:
            maskc_sb = [persist.tile([128, S], F16, tag=f"mask{t}", name=f"mask{t}")
                        for t in range(16)]

            # col = 2048 g + 128 t + r  (q''-order, group-major)
            qt_all = persist.tile([128, 2 * S], F16, tag="qt", name="qt")
            kt_all = persist.tile([128, 2 * S], F16, tag="kt", name="kt")
            qt = [qt_all[:, S * g:S * (g + 1)] for g in range(2)]
            kt = [kt_all[:, S * g:S * (g + 1)] for g in range(2)]
            vaug = [persist.tile([128, 2048], F16, tag=f"vaug{p}", name=f"vaug{p}")
                    for p in range(N_PAIRS)]
            # per-t blocks are [ones (64) | V_t (64)]: denominator rows land
            # at psO partitions 0:63 (reciprocal_approx_fast needs base 0)
            for p in range(N_PAIRS):
                va3 = vaug[p][:, :].rearrange("p (t c) -> p t c", c=128)
                nc.vector.memset(va3[:, :, 0:64], 1.0)
            # stack3[p]: col = 128 u + r; partitions 0:63 = O(t=u) dh,
            # 64:127 = O(t=u+8) dh  (qh0 / qh1 halves of the fold)
            stack3 = [persist.tile([128, 1024], F16, tag=f"stk{p}", name=f"stk{p}")
                      for p in range(N_PAIRS)]
            wo_sb = [persist.tile([128, D], F16, tag=f"wo{u}", name=f"wo{u}")
                     for u in range(8)]
            dumm = persist.tile([128, 8], F32, tag="dumm", name="dumm")

            # warm the exp table set while the PE does assembly
            nc.scalar.activation(dumm[:, :], dumm[:, :],
                                 mybir.ActivationFunctionType.Exp, scale=1.0)

            mask_issued = [False] * 16

            def mask_load(t):
                if not mask_issued[t]:
                    mask_issued[t] = True
                    nc.sync.dma_start(out=maskc_sb[t][:, :],
                                      in_=maskc_d[t::16, :])

            with tc.tile_pool(name="outb", bufs=1) as outpool, \
                 tc.tile_pool(name="pr_pool", bufs=5) as prpool, \
                 tc.tile_pool(name="pm_pool", bufs=12) as pmpool, \
                 tc.tile_pool(name="asm_mt", bufs=12) as mtpool, \
                 tc.tile_pool(name="asm_tmp", bufs=3) as tmppool:

                # ---------------- projections (M=128 psum tiles) ---------
                # psum tile u covers w-cols 128u..128u+128 = fold blocks
                # (2u, 2u+1): partitions 0:63 = dh@t=2u, 64:127 = dh@t=2u+1;
                # cols = 512 x-rows = 4 pair blocks (g,hp).
                def scatter_qk(ps, dst_all, u):
                    def psv(a):  # [64, 4, 128] view of partition half a
                        return ps[64 * a:64 * (a + 1), :].rearrange(
                            "p (q r) -> p q r", r=128)

                    def dstv(hp, t):  # [64, 2, 128] dst: both g, block t
                        return dst_all[64 * hp:64 * (hp + 1),
                                       :].rearrange("p (g c) -> p g c",
                                                    g=2)[:, :, 128 * t:128 * (t + 1)]

                    tA, tB = 2 * u, 2 * u + 1
                    # same-partition pieces (t-parity == hp)
                    nc.vector.tensor_copy(dstv(0, tA), psv(0)[:, 0::2, :])
                    nc.vector.tensor_copy(dstv(1, tB), psv(1)[:, 1::2, :])
                    # cross pieces via fp16 tmp + partition-move DMA
                    tmp = tmppool.tile([128, 256], F16, tag="xt", name="xt")
                    t3a = tmp[0:64, :].rearrange("p (q r) -> p q r", r=128)
                    t3b = tmp[64:128, :].rearrange("p (q r) -> p q r", r=128)
                    nc.vector.tensor_copy(t3a, psv(0)[:, 1::2, :])
                    nc.gpsimd.dma_start(out=dstv(1, tA), in_=t3a)
                    nc.vector.tensor_copy(t3b, psv(1)[:, 0::2, :])
                    nc.gpsimd.dma_start(out=dstv(0, tB), in_=t3b)

                # Q assembly first (attention needs all of qt)
                mts_q = []
                for k in range(8):
                    mt = mtpool.tile([128, 1536], F16, tag="mt", name="mtq")
                    nc.sync.dma_start(out=mt[:, :], in_=qasm[k])
                    mts_q.append(mt)
                mts_k = []
                for k in range(8):
                    mt = mtpool.tile([128, 1536], F16, tag="mt", name="mtk")
                    nc.sync.dma_start(out=mt[:, :], in_=kasm[k])
                    mts_k.append(mt)
                for t in range(MASK_UPFRONT):
                    mask_load(t)

                with tc.tile_pool(name="asm_psq", bufs=8, space="PSUM") as apsq:
                    ps8 = [apsq.tile([128, 512], F32, tag="asm", name="asmps")
                           for _ in range(8)]
                    for k in range(8):
                        for u in range(8):
                            nc.tensor.matmul(
                                ps8[u][:, :],
                                lhsT=mts_q[k][:, 128 * u:128 * (u + 1)],
                                rhs=mts_q[k][:, 1024:1536],
                                start=(k == 0), stop=(k == 7),
                            )
                    for u in range(8):
                        scatter_qk(ps8[u], qt_all, u)

                # ---------------- attention pipeline state ---------------
                flat = [(p, qh, t) for p in range(N_PAIRS)
                        for qh in range(2) for t in range(16)]
                queue = []      # S-emitted steps awaiting PV
                extra = []      # deferred PE work (final-projection units)
                dve_extra = []  # deferred DVE work (normalize closures)
                psO_map = {}
                step_no = [0]

                with tc.tile_pool(name="st_ps", bufs=2, space="PSUM") as stpsum:

                    def emit_s_step():
                        p, qh, t = flat[step_no[0]]
                        # stagger the remaining mask loads ~5 steps ahead
                        mask_load(min(step_no[0] + MASK_UPFRONT, 15))
                        step_no[0] += 1
                        g, hp = p // 2, p % 2
                        lo, hi = 64 * hp, 64 * (hp + 1)
                        stt = stpsum.tile([128, 1024], F32, tag="st", name="stt")
                        for sc in range(2):
                            nc.tensor.matmul(
                                stt[:, 512 * sc:512 * (sc + 1)],
                                lhsT=kt[g][lo:hi, 128 * t:128 * (t + 1)],
                                rhs=qt[g][lo:hi,
                                          1024 * qh + 512 * sc:
                                          1024 * qh + 512 * (sc + 1)],
                                start=True, stop=True,
                            )
                        praw = prpool.tile([128, 1024], F16, tag="praw",
                                           name="praw")
                        nc.scalar.activation(praw[:, :], stt[:, :],
                                             mybir.ActivationFunctionType.Exp,
                                             scale=EXP_SCALE)
                        pm = pmpool.tile([128, 1024], F16, tag="pm", name="pm")
                        nc.vector.tensor_mul(
                            pm[:, :], praw[:, :],
                            maskc_sb[t][:, 1024 * qh:1024 * (qh + 1)])
                        queue.append((p, qh, t, pm))

                    # K assembly in two 4-bank passes, early steps between
                    with tc.tile_pool(name="asm_psk", bufs=4,
                                      space="PSUM") as apsk:
                        for half in range(2):
                            ps4 = [apsk.tile([128, 512], F32, tag="asmk",
                                             name="asmk") for _ in range(4)]
                            for k in range(8):
                                for ui in range(4):
                                    u = 4 * half + ui
                                    nc.tensor.matmul(
                                        ps4[ui][:, :],
                                        lhsT=mts_k[k][:, 128 * u:128 * (u + 1)],
                                        rhs=mts_k[k][:, 1024:1536],
                                        start=(k == 0), stop=(k == 7),
                                    )
                            for ui in range(4):
                                scatter_qk(ps4[ui], kt_all, 4 * half + ui)
                            # steps 0-2 need kt blocks 0-5, covered by
                            # pass 1 (u-tiles 0-3 = blocks 0-7)
                            for _ in range(3):
                                emit_s_step()

                    mtv = []
                    for k in range(8):
                        mt = mtpool.tile([128, 1536], F16, tag="mt", name="mtv")
                        nc.sync.dma_start(out=mt[:, :], in_=vasm[k])
                        mtv.append(mt)
                    for u in range(8):
                        nc.sync.dma_start(out=wo_sb[u][:, :], in_=wo3_d[u])

                    def v_scatter(ps, p, oc):
                        src = ps[:, :].rearrange("p (t c) -> p t c", c=64)
                        dst3 = vaug[p][:, :].rearrange("p (t c) -> p t c", c=128)
                        nc.vector.tensor_copy(
                            dst3[:, 8 * oc:8 * (oc + 1), 64:128], src)

                    # V projection, 8 one-bank groups, early steps between
                    with tc.tile_pool(name="v_ps", bufs=4, space="PSUM") as vpsum:
                        for p in range(N_PAIRS):
                            for oc in range(2):
                                psv = vpsum.tile([128, 512], F32, tag="psv",
                                                 name="psv")
                                for k in range(8):
                                    nc.tensor.matmul(
                                        psv[:, :],
                                        lhsT=mtv[k][:, 1024 + 128 * p:
                                                    1024 + 128 * (p + 1)],
                                        rhs=mtv[k][:, 512 * oc:512 * (oc + 1)],
                                        start=(k == 0), stop=(k == 7),
                                    )
                                v_scatter(psv, p, oc)
                                if step_no[0] < N_EARLY and (p + oc) % 2 == 1:
                                    emit_s_step()

                    with tc.tile_pool(name="norm", bufs=2) as npool, \
                         tc.tile_pool(name="o_ps", bufs=2, space="PSUM") as opsum:

                        def final_unit(p, i):
                            # i = 2u + oc
                            if i == 0:
                                psO_map[("F", p)] = opsum.tile(
                                    [128, 1024], F32, tag="o", name="psF")
                            psF = psO_map[("F", p)]
                            u, oc = i // 2, i % 2
                            nc.tensor.matmul(
                                psF[:, 512 * oc:512 * (oc + 1)],
                                lhsT=stack3[p][:, 128 * u:128 * (u + 1)],
                                rhs=wo_sb[u][:, 512 * oc:512 * (oc + 1)],
                                start=(u == 0), stop=(u == 7),
                            )
                            if i == 15:
                                psF = psO_map.pop(("F", p))
                                ob = outpool.tile([128, D], F32, tag="ob",
                                                  name="ob")
                                nc.vector.tensor_copy(ob[:, :], psF[:, :])
                                nc.gpsimd.dma_start(
                                    out=out_d[128 * p:128 * (p + 1), :],
                                    in_=ob[:, :])

                        def normalize(p, qh, psO):
                            recip = npool.tile([64, 1024], F32, tag="recip",
                                               name="recip")
                            nc.vector.reciprocal_approx_fast(recip[:, :],
                                                             psO[0:64, :])
                            if qh == 0:
                                tmpa = npool.tile([128, 1024], F16, tag="tmpa",
                                                  name="tmpa")
                                nc.vector.tensor_mul(tmpa[64:128, :],
                                                     psO[64:128, :],
                                                     recip[:, :])
                                nc.gpsimd.dma_start(out=stack3[p][0:64, :],
                                                    in_=tmpa[64:128, :])
                            else:
                                nc.vector.tensor_mul(stack3[p][64:128, :],
                                                     psO[64:128, :],
                                                     recip[:, :])
                                for i in range(16):
                                    extra.append(
                                        lambda p=p, i=i: final_unit(p, i))

                        def drain_one():
                            p, qh, t, pm = queue.pop(0)
                            if t == 0:
                                psO_map[(p, qh)] = opsum.tile(
                                    [128, 1024], F32, tag="o", name="psO")
                            psO = psO_map[(p, qh)]
                            for sc in range(2):
                                nc.tensor.matmul(
                                    psO[:, 512 * sc:512 * (sc + 1)],
                                    lhsT=vaug[p][:, 128 * t:128 * (t + 1)],
                                    rhs=pm[:, 512 * sc:512 * (sc + 1)],
                                    start=(t == 0), stop=(t == 15),
                                )
                            if t == 15:
                                psO = psO_map.pop((p, qh))
                                dve_extra.append(
                                    lambda p=p, qh=qh, psO=psO:
                                    normalize(p, qh, psO))

                        while step_no[0] < len(flat):
                            emit_s_step()
                            drained = 0
                            while len(queue) > LAG and drained < 2:
                                drain_one()
                                drained += 1
                            if dve_extra:
                                dve_extra.pop(0)()
                            for _ in range(2):
                                if extra:
                                    extra.pop(0)()
                        while queue:
                            drain_one()
                        while dve_extra:
                            dve_extra.pop(0)()
                        while extra:
                            extra.pop(0)()

    nc.finalize()
    return nc


def build_in_maps(inputs):
    q = np.asarray(inputs["q"], dtype=np.float32)
    k = np.asarray(inputs["k"], dtype=np.float32)
    v = np.asarray(inputs["v"], dtype=np.float32)
    mask = np.asarray(inputs["mask"])
    w_q = np.asarray(inputs["w_q"], dtype=np.float32)
    w_k = np.asarray(inputs["w_k"], dtype=np.float32)
    w_v = np.asarray(inputs["w_v"], dtype=np.float32)
    w_o = np.asarray(inputs["w_o"], dtype=np.float32)

    wqT = np.ascontiguousarray(w_q.T).astype(np.float16).reshape(8, 128, D)
    wkT = np.ascontiguousarray(w_k.T).astype(np.float16).reshape(8, 128, D)
    wvT = np.ascontiguousarray(w_v.T).astype(np.float16).reshape(8, 128, D)
    woT = np.ascontiguousarray(w_o.T)
    wo3 = np.stack([np.concatenate([woT[64 * u:64 * u + 64],
                                    woT[512 + 64 * u:512 + 64 * u + 64]], axis=0)
                    for u in range(8)]).astype(np.float16)
    # St rows are k'; columns are q'' = 128 t + r (permuted query order):
    # maskc[k', 128 t + r] = 1 - mask[b][q' = 16 r + t, k']
    maskc = []
    for b in range(B):
        mt_ = (~mask[b]).T.astype(np.float16)          # [k', q']
        mp = mt_.reshape(S, 128, 16).transpose(0, 2, 1).reshape(S, S)
        maskc.append(np.ascontiguousarray(mp))

    in_maps = []
    for c in range(N_CORES):
        b, sb = c // 4, c % 4
        rows = slice(CORE_ROWS * sb, CORE_ROWS * (sb + 1))
        xqT = np.ascontiguousarray(q[b, rows].T).astype(np.float16).reshape(8, 128, CORE_ROWS)
        xkT = np.ascontiguousarray(k[b, rows].T).astype(np.float16).reshape(8, 128, CORE_ROWS)
        xvT = np.ascontiguousarray(v[b, rows].T).astype(np.float16).reshape(8, 128, CORE_ROWS)
        in_maps.append({
            "qasm": np.concatenate([wqT, xqT], axis=2),
            "kasm": np.concatenate([wkT, xkT], axis=2),
            "vasm": np.concatenate([wvT, xvT], axis=2),
            "wo3": wo3,
            "maskc": maskc[b],
        })
    return in_maps


def kernel(q, k, v, mask, w_q, w_k, w_v, w_o):
    global _NC
    if _NC is None:
        _NC = _build_program()

    in_maps = build_in_maps(dict(q=q, k=k, v=v, mask=mask,
                                 w_q=w_q, w_k=w_k, w_v=w_v, w_o=w_o))
    res = run_bass_kernel_spmd(_NC, in_maps, list(range(N_CORES))).results

    out = np.empty((B, S, D), dtype=np.float32)
    for c in range(N_CORES):
        b, sb = c // 4, c % 4
        out[b, CORE_ROWS * sb:CORE_ROWS * (sb + 1)] = res[c]["out"]
    return out
